# revision 11
# baseline (speedup 1.0000x reference)
"""Trainium2 Bass kernel for nn_BayesianFlowNetworkDiscretised.

Computes, for each (b, d) position:
    MLP: h = gelu_tanh(W1[0,:]*mu + t*W1[1,:] + b1); (mu_eps, ln_sig) = h@W2 + b2
    mu_x = mu/gamma - var_scale*mu_eps
    sigma = max(var_scale*exp(ln_sig), 0.02)   [clip never binds for this data]
    out_k = Phi((e_k - mu_x)/sigma) - Phi((e_{k-1} - mu_x)/sigma),  e_i = i/8 - 1

Sharding: D split across 8 cores (data-parallel, no comm).
Per-core layout: partition p = b*4 + q holds mu[b, q*1536 : (q+1)*1536];
all per-b constants become per-partition [128,1] scale/bias vectors.

dtypes: fp16 for h / MLP accumulators / inv / erf outputs (error-analysed
safe: beta*inv <= ~1 bounds amplification); fp32 for mu, mu_x, final out.
"""

import sys

sys.path.insert(0, "/opt/trn_rl_repo")

import numpy as np

import concourse.bass as bass
import concourse.bacc as bacc
from concourse import mybir
from concourse.tile import TileContext
from concourse.bass_utils import run_bass_kernel_spmd

F32 = mybir.dt.float32
F16 = mybir.dt.float16
AF = mybir.ActivationFunctionType
OP = mybir.AluOpType

K = 16
SIGMA_ONE = 0.02
T_MIN = 1e-6
B, D, H = 32, 49152, 16
NCORES = 8
DS = D // NCORES          # 6144 columns per core
Q = 4                     # partitions per batch row
F = DS // Q               # 1536 free elements per partition
NCHUNK = 4                # output staging chunks
FC = F // NCHUNK          # 512
LN_SQRT2 = 0.34657359027997264


def _build(W1, b1, W2, b2):
    """Build the Bass module. Weights are baked in as immediates."""
    nc = bacc.Bacc(None, target_bir_lowering=False)
    mu_p = nc.declare_dram_parameter("mu", [B, DS], F32, isOutput=False)
    cn_p = nc.declare_dram_parameter("cn", [128, H + 8], F32, isOutput=False)
    out_p = nc.declare_dram_parameter("out", [B, DS, K], F32, isOutput=True)

    mu_v = mu_p.rearrange("b (q f) -> (b q) f", q=Q)
    out_v = out_p.rearrange("b (q c f) k -> (b q) c (f k)", q=Q, c=NCHUNK)

    with TileContext(nc) as tc:
        with (
            tc.tile_pool(name="const", bufs=1) as constp,
            tc.tile_pool(name="main", bufs=1) as mainp,
            tc.tile_pool(name="hp", bufs=16) as hp,
            tc.tile_pool(name="dp", bufs=4) as dp,
            tc.tile_pool(name="fp", bufs=15) as fpool,
            tc.tile_pool(name="op", bufs=2) as opool,
        ):
            cn = constp.tile([128, H + 8], F32)
            nc.sync.dma_start(out=cn[:, :], in_=cn_p[:, :])
            cb = cn[:, 0:H]
            pb = cn[:, H : H + 8]
            mu = mainp.tile([128, F], F32)
            nc.sync.dma_start(out=mu[:, :], in_=mu_v)

            # ACT instructions support a single sync-wait slot; make the ACT
            # engine observe the const-DMA semaphore via a tiny copy so the
            # first gelu only needs to wait on the mu DMA.
            warm = constp.tile([128, 1], F32)
            nc.scalar.copy(out=warm[:, :], in_=cn[:, 0:1])

            alpha = pb[:, 0:1]      # 1/gamma            (0 if cond)
            negbeta = pb[:, 1:2]    # -var_scale         (0 if cond)
            lnA = pb[:, 2:3]        # ln(var_scale)      (-1e4 if cond)
            lnm = pb[:, 3:4]        # ln(0.02)           (0 if cond)
            nb20 = pb[:, 4:5]       # -var_scale*b2[0]   (0 if cond)

            # ---- phase A: MLP (gelu on ACT, fused mul-add accumulate on DVE)
            acc_e = mainp.tile([128, F], F16)   # sum_j W2[j,0]*h_j + b2[0]
            acc_l = mainp.tile([128, F], F16)   # sum_j W2[j,1]*h_j + b2[1]
            for j in range(H):
                h = hp.tile([128, F], F16)
                nc.scalar.activation(
                    out=h, in_=mu, func=AF.Gelu_apprx_tanh,
                    bias=cb[:, j : j + 1], scale=float(W1[0, j]),
                )
                if j == 0:
                    nc.vector.tensor_scalar(
                        out=acc_e, in0=h, scalar1=float(W2[0, 0]),
                        scalar2=float(b2[0]), op0=OP.mult, op1=OP.add)
                    nc.vector.tensor_scalar(
                        out=acc_l, in0=h, scalar1=float(W2[0, 1]),
                        scalar2=float(b2[1]), op0=OP.mult, op1=OP.add)
                else:
                    nc.vector.scalar_tensor_tensor(
                        out=acc_e, in0=h, scalar=float(W2[j, 0]), in1=acc_e,
                        op0=OP.mult, op1=OP.add)
                    nc.vector.scalar_tensor_tensor(
                        out=acc_l, in0=h, scalar=float(W2[j, 1]), in1=acc_l,
                        op0=OP.mult, op1=OP.add)

            # ---- mu_x = alpha*mu - beta*acc_e   (acc_e already includes b2[0])
            mx = mainp.tile([128, F], F32)
            nc.vector.tensor_scalar_mul(out=mx, in0=mu, scalar1=alpha)
            nc.vector.scalar_tensor_tensor(
                out=mx, in0=acc_e, scalar=negbeta, in1=mx,
                op0=OP.mult, op1=OP.add)

            # ---- phase B: inv = 1/(sqrt(2)*sigma) = exp(-max(lnsig+lnA, lnm) - ln(sqrt 2))
            v = mainp.tile([128, F], F16)
            nc.vector.tensor_scalar(
                out=v, in0=acc_l, scalar1=lnA, scalar2=lnm,
                op0=OP.add, op1=OP.max)
            inv = mainp.tile([128, F], F16)
            nc.scalar.activation(
                out=inv, in_=v, func=AF.Exp, scale=-1.0, bias=pb[:, 5:6])

            # ---- phase C: f_i = 0.5*erf((e_i - mu_x)*inv), i = 1..15
            fts = []
            for i in range(1, 16):
                e_i = i / 8.0 - 1.0
                dt_ = dp.tile([128, F], F16)
                nc.vector.tensor_scalar(
                    out=dt_, in0=mx, scalar1=-1.0, scalar2=float(e_i),
                    op0=OP.mult, op1=OP.add)
                nc.vector.tensor_mul(out=dt_, in0=dt_, in1=inv)
                fi = fpool.tile([128, F], F16)
                nc.scalar.activation(out=fi, in_=dt_, func=AF.Erf)
                nc.vector.tensor_scalar_mul(out=fi, in0=fi, scalar1=0.5)
                fts.append(fi)

            # ---- diffs into k-interleaved staging, then DMA out
            for ci in range(NCHUNK):
                sl = slice(ci * FC, (ci + 1) * FC)
                o = opool.tile([128, FC, K], F32)
                nc.vector.tensor_scalar_add(
                    out=o[:, :, 0], in0=fts[0][:, sl], scalar1=0.5)
                for k2 in range(1, 15):
                    nc.gpsimd.tensor_tensor(
                        out=o[:, :, k2], in0=fts[k2][:, sl],
                        in1=fts[k2 - 1][:, sl], op=OP.subtract)
                nc.vector.tensor_scalar(
                    out=o[:, :, 15], in0=fts[14][:, sl], scalar1=-1.0,
                    scalar2=0.5, op0=OP.mult, op1=OP.add)
                nc.sync.dma_start(
                    out=out_v[:, ci, :], in_=o[:, :, :].rearrange("p a b -> p (a b)"))

    return nc


def _host_consts(t, W1, b1, W2, b2):
    t = np.asarray(t, np.float64).reshape(B)
    cond = t < T_MIN
    gamma = 1.0 - SIGMA_ONE ** (2.0 * t)
    alpha = np.where(cond, 0.0, 1.0 / gamma)
    beta = np.sqrt(np.maximum(1.0 - gamma, 0.0) / gamma)
    negbeta = np.where(cond, 0.0, -beta)
    lnA = np.where(cond, -1e4, np.log(np.maximum(beta, 1e-300)))
    lnm = np.where(cond, 0.0, np.log(SIGMA_ONE))
    nb20 = np.where(cond, 0.0, -beta * float(b2[0]))

    pb = np.zeros((128, 8), np.float32)
    for b in range(B):
        for q in range(Q):
            p = b * Q + q
            pb[p, 0] = alpha[b]
            pb[p, 1] = negbeta[b]
            pb[p, 2] = lnA[b]
            pb[p, 3] = lnm[b]
            pb[p, 4] = nb20[b]
            pb[p, 5] = -LN_SQRT2

    cb = np.zeros((128, H), np.float32)
    cvals = t[:, None] * np.asarray(W1, np.float64)[1, :][None, :] + np.asarray(
        b1, np.float64)[None, :]                        # [B, H]
    for b in range(B):
        cb[b * Q : (b + 1) * Q, :] = cvals[b]
    return cb, pb


def _run(inputs, trace=False):
    mu = np.ascontiguousarray(np.asarray(inputs["mu"], np.float32))
    t = np.asarray(inputs["t"], np.float32)
    W1 = np.asarray(inputs["W1"], np.float32)
    b1 = np.asarray(inputs["b1"], np.float32)
    W2 = np.asarray(inputs["W2"], np.float32)
    b2 = np.asarray(inputs["b2"], np.float32)

    nc = _build(W1, b1, W2, b2)
    nc.finalize()
    cb, pb = _host_consts(t, W1, b1, W2, b2)

    cn = np.ascontiguousarray(np.concatenate([cb, pb], axis=1))
    in_maps = []
    for c in range(NCORES):
        shard = np.ascontiguousarray(mu[:, c * DS : (c + 1) * DS])
        in_maps.append({"mu": shard, "cn": cn})

    res = run_bass_kernel_spmd(nc, in_maps, list(range(NCORES)), trace=trace)
    out = np.concatenate(
        [np.asarray(res.results[c]["out"]) for c in range(NCORES)], axis=1)
    return out, res


def kernel(**inputs) -> np.ndarray:
    out, _ = _run(inputs, trace=False)
    return out


if __name__ == "__main__":
    rng = np.random.default_rng(0)
    demo = {
        "mu": rng.standard_normal((B, D), dtype=np.float32),
        "t": rng.random((B, 1), dtype=np.float32),
        "W1": rng.standard_normal((2, H), dtype=np.float32) * 0.5,
        "b1": rng.standard_normal((H,), dtype=np.float32) * 0.1,
        "W2": rng.standard_normal((H, 2), dtype=np.float32) * 0.1,
        "b2": rng.standard_normal((2,), dtype=np.float32) * 0.1,
    }
    out = kernel(**demo)
    print("kernel output", out.shape, out.dtype, out[0, 0])
